# revision 25
# baseline (speedup 1.0000x reference)
"""Trainium2 Bass kernel for BasicMotionEncoder (RAFT motion encoder).

Network (all stride-1 convs, NCHW, fp32):
    cor  = relu(conv1x1(corr, wc1, bc1))          # [B,256,H,W]
    cor  = relu(conv3x3(cor,  wc2, bc2, pad 1))   # [B,192,H,W]
    flo  = relu(conv7x7(flow, wf1, bf1, pad 3))   # [B,128,H,W]
    flo  = relu(conv3x3(flo,  wf2, bf2, pad 1))   # [B,64,H,W]
    out  = relu(conv3x3(cat(cor,flo), wo, bo, 1)) # [B,126,H,W]
    return cat(out, flow)                         # [B,128,H,W]

Sharding: pure data parallel, one image per NeuronCore (B=8, 8 cores).
Each core processes its image in three 32-row passes (with halo
recompute) so every intermediate activation stays in SBUF; only corr is
streamed in and the 126-channel output written back.  Convs are PE
matmuls with channels on the partition dim: for each tap the shifted
input window is a strided AP into a zero-padded SBUF image, accumulated
in PSUM over taps and k-tiles (float32r operands, fp32 PSUM).  All
SBUF tiles live in pools opened once for the whole kernel, so cross-
pass reuse is tracked at byte-range granularity and passes pipeline
into each other.  The final concat of `flow` into channels 126:128
happens on the host.
"""

import numpy as np

import concourse.mybir as mybir
import concourse.tile as tile
from concourse import bacc
from concourse.bass_utils import run_bass_kernel_spmd

H, W = 96, 128
CIN_CORR = 324
WP = W + 2  # pad-1 padded row width (3x3 convs)
PR = 32  # output rows per pass
NPASS = H // PR
F32 = mybir.dt.float32
F32R = mybir.dt.float32r  # 1 row/cycle through the PE (plain fp32 is 4)
RELU = mybir.ActivationFunctionType.Relu

NR_CAT = PR + 2  # catpad rows    [o0-1, o1+1)
NR_C1 = PR + 4  # cor1/flo1 rows  [o0-2, o1+2)
ZELEMS = WP  # zeros tensor length (seeds the SBUF zeros tile)


def _block_starts(lo, hi):
    """4-row block starts covering [lo, hi); the last block is shifted back
    so every block is exactly 4 rows (overlap rows are recomputed)."""
    starts = list(range(lo, hi - 3, 4))
    if starts[-1] != hi - 4:
        starts.append(hi - 4)
    return starts


COPY = mybir.ActivationFunctionType.Copy


def _zero_borders(nc, zsb, buf, zrow):
    """Zero the conv-padding bytes of a padded image buffer with small ACT
    copies from an SBUF zeros tile: cols {0,1} and {128,129} of every row
    (cols 1/128 are interior and overwritten by the relu writes that
    follow), plus the one vertical-padding zero row an edge pass reads.
    memset has no fp32r encoding, strided DMA fills are pathologically
    slow in the DGEs, and bulk zero-fill DMAs steal enough SBUF write
    bandwidth to slow the PE's operand streaming — ACT copies of ~300
    bytes/partition avoid all three."""
    nr = buf.shape[1]
    zv = zsb[:, 0 : 2 * nr].rearrange("p (a b) -> p a b", b=2)
    for off in (0, W):
        nc.scalar.activation(buf[:, :, off : off + 2], zv, COPY)
    if zrow is not None:
        nc.scalar.activation(
            buf[:, zrow : zrow + 1, :],
            zsb[:, 0:WP].rearrange("p (a b) -> p a b", a=1),
            COPY,
        )


def _emit_pass(nc, tc, pools, ws, p, finish_setup):
    """Emit one pass (output rows [PR*p, PR*p+PR))."""
    pspool, apool, spool, opool = pools
    corr, stackh, zsb, out, wc1s, wc2s, wf1s, wf2s, wos, bs, early_setup = ws
    o0, o1 = PR * p, PR * p + PR
    C0, C1 = max(0, o0 - 1), min(H, o1 + 1)  # cat rows computed
    R0, R1 = max(0, o0 - 2), min(H, o1 + 2)  # cor1/flo1 rows computed
    cat_b = o0 - 1  # cat row held in catpad row 0
    cp_b = o0 - 2  # cor1/flo1 row held in *pad row 0

    # 7x7-conv input: full host-side im2col — partition cin*49+dh*7+dw
    # holds the zero-padded flow image shifted by (dh, dw), so one K=98
    # matmul computes a whole f1 block.  Chunked so the first f1 block
    # isn't gated on the whole transfer.
    stack2 = apool.tile([98, NR_C1, 128], F32R, tag="stack2", name=f"stack2_{p}")
    nc.sync.dma_start(out=stack2[:, 0:8, :], in_=stackh[:, o0 : o0 + 8, :])
    if p == 0:
        early_setup()
    for a, b in ((8, 20), (20, NR_C1)):
        nc.sync.dma_start(out=stack2[:, a:b, :], in_=stackh[:, o0 + a : o0 + b, :])
    flo1 = apool.tile([128, NR_C1, WP], F32R, tag="flo1", name=f"flo1_{p}")
    _zero_borders(nc, zsb, flo1, {0: 1, NPASS - 1: NR_C1 - 2}.get(p))
    if p == 0:
        finish_setup()

    # --- f1: 7x7 conv, 2 -> 128 channels ---
    # relu emitted in 2-row halves so f2's row-range dependencies release
    # earlier during the pass-0 ramp
    for rr in _block_starts(R0, R1):
        ps = pspool.tile([128, 4, 128], F32, tag="ps", name=f"psf1_{p}_{rr}")
        i = rr - cp_b
        nc.tensor.matmul(ps, wf1s, stack2[:, i : i + 4, :], start=True, stop=True)
        for h in (0, 2):
            nc.scalar.activation(
                flo1[:, i + h : i + h + 2, 1 : 1 + W],
                ps[:, h : h + 2, :],
                RELU,
                bias=bs[:, 4:5],
            )

    # --- f2: 3x3 conv, 128 -> 64 channels -> catpad2[64:128] ---
    # fp32r matmuls reject a column tile_position, so accumulate at psum
    # partitions 0:64 and partition-shift into catpad2[64:128] with an
    # SBUF->SBUF DMA.
    catpad2 = apool.tile([128, NR_CAT, WP], F32R, tag="catpad2", name=f"catpad2_{p}")
    _zero_borders(nc, zsb, catpad2, {0: 0, NPASS - 1: NR_CAT - 1}.get(p))
    def emit_f2(cc):
        ps = pspool.tile([128, 4, 128], F32, tag="ps", name=f"psf2_{p}_{cc}")
        k = 0
        for dh in range(3):
            for dw in range(3):
                i = cc - cp_b - 1 + dh
                nc.tensor.matmul(
                    ps[0:64],
                    wf2s[:, k, :],
                    flo1[:, i : i + 4, dw : dw + 128],
                    start=(k == 0),
                    stop=(k == 8),
                )
                k += 1
        flo2t = apool.tile(
            [64, 4, 128], F32R, tag="flo2t", bufs=3, name=f"flo2t_{p}_{cc}"
        )
        nc.scalar.activation(flo2t, ps[0:64], RELU, bias=bs[0:64, 5:6])
        nc.scalar.dma_start(
            out=catpad2[64:128, cc - cat_b : cc - cat_b + 4, 1 : 1 + W], in_=flo2t
        )

    # --- c1 / c2 / o, interleaved ---
    # c1 has only 6 matmuls per block against ~2us of corr DMA, so run on
    # its own it starves the PE (and the HAM clock gate re-throttles).
    # Interleaving c2 (36 matmuls/block) and o (18) behind it keeps the PE
    # dense while corr streams.
    cor1a = apool.tile([128, NR_C1, WP], F32R, tag="cor1a", name=f"cor1a_{p}")
    cor1b = apool.tile([128, NR_C1, WP], F32R, tag="cor1b", name=f"cor1b_{p}")
    for buf in (cor1a, cor1b):
        _zero_borders(nc, zsb, buf, {0: 1, NPASS - 1: NR_C1 - 2}.get(p))
    catpad1 = apool.tile([128, NR_CAT, WP], F32R, tag="catpad1", name=f"catpad1_{p}")
    _zero_borders(nc, zsb, catpad1, {0: 0, NPASS - 1: NR_CAT - 1}.get(p))

    def emit_c1_dma(rr):
        cts = []
        for kt in range(3):
            kk = 128 if kt < 2 else CIN_CORR - 256
            ct = spool.tile([128, 4, 128], F32R, tag="corr", name=f"ct_{p}_{rr}_{kt}")
            nc.sync.dma_start(
                out=ct[0:kk], in_=corr[kt * 128 : kt * 128 + kk, rr : rr + 4, :]
            )
            cts.append((ct, kk))
        return cts

    def emit_c1_mm(rr, cts):
        ps0 = pspool.tile([128, 4, 128], F32, tag="ps", name=f"psc1a_{p}_{rr}")
        ps1 = pspool.tile([128, 4, 128], F32, tag="ps", name=f"psc1b_{p}_{rr}")
        for kt, (ct, kk) in enumerate(cts):
            nc.tensor.matmul(
                ps0, wc1s[0:kk, kt, 0:128], ct[0:kk], start=(kt == 0), stop=(kt == 2)
            )
            nc.tensor.matmul(
                ps1, wc1s[0:kk, kt, 128:256], ct[0:kk], start=(kt == 0), stop=(kt == 2)
            )
        r = rr - cp_b
        nc.scalar.activation(cor1a[:, r : r + 4, 1 : 1 + W], ps0, RELU, bias=bs[:, 0:1])
        nc.scalar.activation(cor1b[:, r : r + 4, 1 : 1 + W], ps1, RELU, bias=bs[:, 1:2])

    def emit_c2(cc):
        ps0 = pspool.tile([128, 4, 128], F32, tag="ps", name=f"psc2a_{p}_{cc}")
        ps1 = pspool.tile([128, 4, 128], F32, tag="ps", name=f"psc2b_{p}_{cc}")
        k = 0
        for kt, src_ in enumerate((cor1a, cor1b)):
            for dh in range(3):
                for dw in range(3):
                    i = cc - cp_b - 1 + dh
                    rhs = src_[:, i : i + 4, dw : dw + 128]
                    tap = 3 * dh + dw
                    nc.tensor.matmul(
                        ps0, wc2s[:, tap, kt, 0:128], rhs, start=(k == 0), stop=(k == 17)
                    )
                    nc.tensor.matmul(
                        ps1[0:64],
                        wc2s[:, tap, kt, 128:192],
                        rhs,
                        start=(k == 0),
                        stop=(k == 17),
                    )
                    k += 1
        r = cc - cat_b
        nc.scalar.activation(catpad1[:, r : r + 4, 1 : 1 + W], ps0, RELU, bias=bs[:, 2:3])
        nc.scalar.activation(
            catpad2[0:64, r : r + 4, 1 : 1 + W], ps1[0:64], RELU, bias=bs[0:64, 3:4]
        )

    def emit_o(oo):
        ps = pspool.tile([128, 4, 128], F32, tag="ps", name=f"pso_{p}_{oo}")
        k = 0
        for kt, src_ in enumerate((catpad1, catpad2)):
            for dh in range(3):
                for dw in range(3):
                    i = oo - cat_b - 1 + dh
                    nc.tensor.matmul(
                        ps[0:126],
                        wos[:, kt, 3 * dh + dw, :],
                        src_[:, i : i + 4, dw : dw + 128],
                        start=(k == 0),
                        stop=(k == 17),
                    )
                    k += 1
        ob = opool.tile([128, 4, 128], F32, tag="ob", name=f"ob_{p}_{oo}")
        nc.scalar.activation(ob[0:126], ps[0:126], RELU, bias=bs[0:126, 6:7])
        nc.scalar.dma_start(out=out[:, oo : oo + 4, :], in_=ob[0:126])

    b1 = _block_starts(R0, R1)
    b2 = _block_starts(C0, C1)
    b3 = list(range(o0, o1, 4))
    PF = 3  # corr DMA issue runs this many blocks ahead of the c1 matmuls
    cts_q = {}
    for idx in range(len(b1) + 4):
        if idx == 0:
            for j in range(PF):
                cts_q[j] = emit_c1_dma(b1[j])
        elif idx + PF - 1 < len(b1):
            cts_q[idx + PF - 1] = emit_c1_dma(b1[idx + PF - 1])
        if idx < len(b2):
            emit_f2(b2[idx])
        if idx < len(b1):
            emit_c1_mm(b1[idx], cts_q.pop(idx))
        if 0 <= idx - 2 < len(b2):
            emit_c2(b2[idx - 2])
        if 0 <= idx - 4 < len(b3):
            emit_o(b3[idx - 4])


def build_module():
    nc = bacc.Bacc(trn_type="TRN2", target_bir_lowering=False)
    corr = nc.dram_tensor("corr", [CIN_CORR, H, W], F32R, kind="ExternalInput").ap()
    stackh = nc.dram_tensor("stackh", [98, H + 4, 128], F32R, kind="ExternalInput").ap()
    zeros = nc.dram_tensor("zeros", [128, ZELEMS], F32R, kind="ExternalInput").ap()
    wc1p = nc.dram_tensor("wc1p", [128, 3, 256], F32R, kind="ExternalInput").ap()
    wc2p = nc.dram_tensor("wc2p", [128, 9, 2, 192], F32R, kind="ExternalInput").ap()
    wf1p = nc.dram_tensor("wf1p", [98, 128], F32R, kind="ExternalInput").ap()
    wf2p = nc.dram_tensor("wf2p", [128, 9, 64], F32R, kind="ExternalInput").ap()
    wop = nc.dram_tensor("wop", [128, 2, 9, 126], F32R, kind="ExternalInput").ap()
    biasp = nc.dram_tensor("biasp", [128, 8], F32, kind="ExternalInput").ap()
    out = nc.dram_tensor("out", [126, H, W], F32, kind="ExternalOutput").ap()

    with tile.TileContext(nc) as tc:
        with (
            tc.tile_pool(name="wpool", bufs=1) as wpool,
            tc.tile_pool(name="pspool", space="PSUM", bufs=8) as pspool,
            tc.tile_pool(name="apool", bufs=1) as apool,
            tc.tile_pool(name="spool", bufs=9) as spool,
            tc.tile_pool(name="opool", bufs=3) as opool,
        ):
            wc1s = wpool.tile([128, 3, 256], F32R, name="wc1s")
            wc2s = wpool.tile([128, 9, 2, 192], F32R, name="wc2s")
            wf1s = wpool.tile([98, 128], F32R, name="wf1s")
            wf2s = wpool.tile([128, 9, 64], F32R, name="wf2s")
            wos = wpool.tile([128, 2, 9, 126], F32R, name="wos")
            bs = wpool.tile([128, 8], F32, name="bs")
            zsb = wpool.tile([128, WP], F32R, name="zsb")
            scr = wpool.tile([128, 1], F32, name="scr")
            # what f1 needs comes first (the pass-0 stack chunk DMA is
            # emitted before these inside _emit_pass); the rest of the
            # weights are DMA'd behind it (finish_setup below)
            def early_setup():
                # scalar HWDGE queue: runs in parallel with the stack2
                # chunk transfers on sync, so the first f1 matmul's two
                # prerequisites don't serialize
                nc.scalar.dma_start(out=wf1s, in_=wf1p)
                nc.scalar.dma_start(out=bs, in_=biasp)
                nc.scalar.dma_start(out=zsb, in_=zeros)
                # prewarm the Relu activation table off the critical path
                nc.scalar.activation(scr, bs[:, 7:8], RELU, bias=bs[:, 7:8])

            def finish_setup():
                for sb, dr in ((wc1s, wc1p), (wc2s, wc2p), (wf2s, wf2p), (wos, wop)):
                    nc.sync.dma_start(out=sb, in_=dr)

            ws = (corr, stackh, zsb, out, wc1s, wc2s, wf1s, wf2s, wos, bs, early_setup)
            pools = (pspool, apool, spool, opool)
            for p in range(NPASS):
                _emit_pass(nc, tc, pools, ws, p, finish_setup)
    nc.compile()
    return nc


def pack_params(wc1, bc1, wc2, bc2, wf1, bf1, wf2, bf2, wo, bo):
    """Host-side repack of OIHW conv weights into the lhsT layouts the
    kernel's matmuls read ([K partitions, ..., M])."""
    f = np.float32
    wc1p = np.zeros((128, 3, 256), f)
    w = wc1[:, :, 0, 0]  # [256, 324]
    for kt in range(3):
        kk = min(128, CIN_CORR - kt * 128)
        wc1p[0:kk, kt, :] = w[:, kt * 128 : kt * 128 + kk].T
    wc2p = np.zeros((128, 9, 2, 192), f)
    for dh in range(3):
        for dw in range(3):
            for kt in range(2):
                wc2p[:, 3 * dh + dw, kt, :] = wc2[:, kt * 128 : kt * 128 + 128, dh, dw].T
    wf1p = np.zeros((98, 128), f)
    for cin in range(2):
        for dh in range(7):
            for dw in range(7):
                wf1p[cin * 49 + dh * 7 + dw, :] = wf1[:, cin, dh, dw]
    wf2p = np.zeros((128, 9, 64), f)
    for dh in range(3):
        for dw in range(3):
            wf2p[:, 3 * dh + dw, :] = wf2[:, :, dh, dw].T
    wop = np.zeros((128, 2, 9, 126), f)
    for dh in range(3):
        for dw in range(3):
            tap = 3 * dh + dw
            wop[:, 0, tap, :] = wo[:, 0:128, dh, dw].T
            wop[0:64, 1, tap, :] = wo[:, 128:192, dh, dw].T
            wop[64:128, 1, tap, :] = wo[:, 192:256, dh, dw].T
    biasp = np.zeros((128, 8), f)
    biasp[:, 0] = bc1[0:128]
    biasp[:, 1] = bc1[128:256]
    biasp[:, 2] = bc2[0:128]
    biasp[0:64, 3] = bc2[128:192]
    biasp[:, 4] = bf1
    biasp[0:64, 5] = bf2
    biasp[0:126, 6] = bo
    return {
        "wc1p": wc1p,
        "wc2p": wc2p,
        "wf1p": wf1p,
        "wf2p": wf2p,
        "wop": wop,
        "biasp": biasp,
    }


def build_stackh(flow_b):
    """Full f1 im2col: [98, H+4, 128], partition cin*49+dh*7+dw holds the
    zero-padded (pad 3) flow image shifted by (dh, dw); row i <-> f1 output
    row i-2."""
    fz = np.zeros((2, H + 10, W + 6), np.float32)
    fz[:, 5 : 5 + H, 3 : 3 + W] = flow_b
    s = np.empty((98, H + 4, 128), np.float32)
    for cin in range(2):
        for dh in range(7):
            for dw in range(7):
                # output row r (= buffer row r+2) col c reads fz row r+dh+2, col c+dw
                s[cin * 49 + dh * 7 + dw] = fz[cin, dh : dh + H + 4, dw : dw + 128]
    return s


_MODULE = None


def _get_module():
    global _MODULE
    if _MODULE is None:
        _MODULE = build_module()
    return _MODULE


def make_in_maps(**inputs):
    a = {
        k: np.ascontiguousarray(np.asarray(v), dtype=np.float32)
        for k, v in inputs.items()
    }
    packed = pack_params(
        a["wc1"], a["bc1"], a["wc2"], a["bc2"], a["wf1"], a["bf1"],
        a["wf2"], a["bf2"], a["wo"], a["bo"],
    )
    zeros = np.zeros((128, ZELEMS), np.float32)
    in_maps = []
    for b in range(8):
        m = dict(packed)
        m["corr"] = np.ascontiguousarray(a["corr"][b])
        m["stackh"] = build_stackh(a["flow"][b])
        m["zeros"] = zeros
        in_maps.append(m)
    return in_maps, a["flow"]


def assemble_output(results, flow):
    out = np.empty((8, 128, H, W), np.float32)
    for b in range(8):
        out[b, :126] = results[b]["out"]
        out[b, 126:] = flow[b]
    return out


def run(trace=False, **inputs):
    in_maps, flow = make_in_maps(**inputs)
    nc = _get_module()
    res = run_bass_kernel_spmd(nc, in_maps, core_ids=list(range(8)), trace=trace)
    return assemble_output(res.results, flow), res


def kernel(**inputs):
    out, _ = run(trace=False, **inputs)
    return out
